# revision 2
# baseline (speedup 1.0000x reference)
"""BEV detection loss on 8 Trainium2 NeuronCores — v2.

Strategy (data-parallel over batch, one batch element per core):
  - Only obj_logits needs a full per-cell scan: sum softplus(x) over all
    262144 cells per batch element.  cls/box terms touch <=64 positive
    cells; that tiny gather + math runs on host (as the pack/combine
    already did in v1).
  - Device computes, per core, from x = obj_logits[b] as bf16 [128,2048]:
        Sr = sum relu(x)            (DVE tensor_scalar max+add accum)
        t  = |x|                    (DVE bitwise_and on uint16 view)
        St = sum tanh(s*t + b)      (ACT, one pass, fused accumulate)
    Then sum softplus(x) = Sr + sum ln(1+exp(-t))
                        ~= Sr + A*count - A*St,
    using the 3-parameter fit ln(1+e^-t) ~= A*(1 - tanh(s*t + b)) with
    max pointwise err 1.2e-3 on t>=0 (tanh saturates for large t, so the
    approximation degrades gracefully) -- final loss rel err ~1e-5 on
    normal data, worst case ~7e-4, far inside the 2e-2 gate.
  - bf16 input halves DMA bytes; ACT runs at the same rate either way,
    DVE gets its 2-byte fast mode.
  - DMA: sync + gpsimd queues for the two input chunks (scalar engine
    stays free so the tanh table load starts immediately), sync for the
    tiny [128,4] output of per-partition partial sums.
"""

import sys

import numpy as np

sys.path.insert(0, "/opt/trn_rl_repo")

import concourse.bacc as bacc  # noqa: E402
import concourse.mybir as mybir  # noqa: E402
import concourse.tile as tile  # noqa: E402
from concourse.bass_utils import run_bass_kernel_spmd  # noqa: E402

import ml_dtypes  # noqa: E402

# BEV grid constants (must match the reference)
X_MIN = np.float32(-51.2)
X_MAX = np.float32(51.2)
Y_MIN = np.float32(-51.2)
Y_MAX = np.float32(51.2)
RES = np.float32(0.2)
BEV_W = 512
BEV_H = 512
NUM_CELLS = BEV_W * BEV_H  # 262144
CLS_WEIGHT = np.float64(1.0)
BOX_WEIGHT = np.float64(1.0)

N_CORES = 8
P_DIM = 128
COLS = NUM_CELLS // P_DIM  # 2048
D = 7

# ln(1+e^-t) ~= A*(1 - tanh(S*t + B)) for t >= 0 (max err 1.2e-3)
TANH_A = 1.220978
TANH_S = 0.494342
TANH_B = 0.463907

# chunk column ranges and their input-DMA queue engines; processed in list
# order by the ACT stream.  Small first chunk on the (fast-start) sync queue
# so tanh work begins early; the gpsimd-queue chunk is needed last.
CHUNKS = [(0, 384), (384, 1024), (1024, 2048)]
DMA_ENGINES = ["sync", "sync", "gpsimd"]
WARM_DMA = False  # tiny leading transfer per queue (measured: no benefit)
OUT_W = 2 * len(CHUNKS)  # relu accums + tanh accums per chunk

_CACHE = {}


def _build_program():
    f32 = mybir.dt.float32
    bf16 = mybir.dt.bfloat16
    u16 = mybir.dt.uint16
    AF = mybir.ActivationFunctionType
    ALU = mybir.AluOpType

    nc = bacc.Bacc("TRN2", debug=False, target_bir_lowering=False, num_devices=N_CORES)
    in_all = nc.dram_tensor("in_all", [P_DIM, COLS], bf16, kind="ExternalInput").ap()
    out_all = nc.dram_tensor("out_all", [P_DIM, OUT_W], f32, kind="ExternalOutput").ap()

    with tile.TileContext(nc) as tc:
        with tc.tile_pool(name="p", bufs=1) as p:
            x = p.tile([P_DIM, COLS], bf16)
            if WARM_DMA:
                wdma = p.tile([P_DIM, 2], bf16)
                nc.sync.dma_start(out=wdma[:, 0:1], in_=in_all[:, 0:1])
                nc.gpsimd.dma_start(out=wdma[:, 1:2], in_=in_all[:, 1:2])
            for (lo, hi), eng in zip(CHUNKS, DMA_ENGINES):
                getattr(nc, eng).dma_start(out=x[:, lo:hi], in_=in_all[:, lo:hi])

            out = p.tile([P_DIM, OUT_W], f32)
            nc.vector.memset(out[:], 0.0)
            bias_t = p.tile([P_DIM, 1], f32)
            nc.vector.memset(bias_t[:], float(TANH_B))

            # data-independent warmup: hoists the tanh table load to block
            # start so it overlaps the input DMA (scale=0 -> input unread)
            warm = p.tile([P_DIM, 1], f32)
            nc.scalar.activation(warm[:], warm[:], AF.Tanh, scale=0.0)

            t = p.tile([P_DIM, COLS], bf16)
            y = p.tile([P_DIM, COLS], bf16)
            nch = len(CHUNKS)
            rdead = [p.tile([P_DIM, hi - lo], bf16, name=f"rd{i}", tag=f"rd{i}")
                     for i, (lo, hi) in enumerate(CHUNKS)]

            # |x| per chunk first: unblocks the ACT tanh stream
            for i, (lo, hi) in enumerate(CHUNKS):
                nc.vector.tensor_scalar(
                    t[:, lo:hi].bitcast(u16), x[:, lo:hi].bitcast(u16),
                    0x7FFF, None, op0=ALU.bitwise_and,
                )
            # St: sum tanh(S*t + B) per chunk, fused accumulate
            for i, (lo, hi) in enumerate(CHUNKS):
                nc.scalar.activation(
                    y[:, lo:hi], t[:, lo:hi], AF.Tanh,
                    scale=float(TANH_S), bias=bias_t[:],
                    accum_out=out[:, nch + i : nch + i + 1],
                )
            # Sr: sum relu(x) per chunk (fused elementwise+reduce)
            for i, (lo, hi) in enumerate(CHUNKS):
                nc.vector.tensor_scalar(
                    rdead[i][:], x[:, lo:hi], 0.0, 0.0, op0=ALU.max, op1=ALU.add,
                    accum_out=out[:, i : i + 1],
                )

            nc.sync.dma_start(out=out_all[:], in_=out[:])

    nc.finalize()
    return nc


def get_program():
    if "nc" not in _CACHE:
        _CACHE["nc"] = _build_program()
    return _CACHE["nc"]


def _assign(gt_boxes, gt_labels, gt_masks):
    """Host-side first-come-wins assignment (mirrors reference float32 math).
    Returns per-batch list of (cell, n) winner pairs."""
    B, N = gt_labels.shape
    gb = np.asarray(gt_boxes, dtype=np.float32)
    x = gb[..., 0]
    y = gb[..., 1]
    in_b = (x >= X_MIN) & (x <= X_MAX) & (y >= Y_MIN) & (y <= Y_MAX)
    gx = np.clip(np.floor((x - X_MIN) / RES).astype(np.int32), 0, BEV_W - 1)
    gy = np.clip(np.floor((y - Y_MIN) / RES).astype(np.int32), 0, BEV_H - 1)
    idx = gy * BEV_W + gx  # [B, N]
    valid = (
        (np.asarray(gt_masks, dtype=np.float32) > 0.5)
        & (np.asarray(gt_labels) >= 0)
        & in_b
    )
    winners = []
    for b in range(B):
        seen = set()
        pairs = []
        for n in range(N):
            if not valid[b, n]:
                continue
            cell = int(idx[b, n])
            if cell in seen:
                continue
            seen.add(cell)
            pairs.append((cell, n))
        winners.append(pairs)
    return winners


def _host_positive_sums(cls_logits, obj_logits, box_preds, gt_boxes, gt_labels,
                        winners):
    """Exact f64 math over the <=64 positive cells per batch element."""
    s_neg = 0.0  # sum softplus(-obj) at positives
    s_pos = 0.0  # sum softplus(obj) at positives
    s_ce = 0.0
    s_box = 0.0
    total_pos = 0
    for b, pairs in enumerate(winners):
        if not pairs:
            continue
        cells = np.array([c for c, _ in pairs])
        ns = np.array([n for _, n in pairs])
        total_pos += len(pairs)
        o = obj_logits[b, cells].astype(np.float64)
        s_neg += np.logaddexp(0.0, -o).sum()
        s_pos += np.logaddexp(0.0, o).sum()
        cl = cls_logits[b, cells].astype(np.float64)  # [P, C]
        m = cl.max(axis=1, keepdims=True)
        lse = m[:, 0] + np.log(np.exp(cl - m).sum(axis=1))
        lbl = np.asarray(gt_labels)[b, ns].astype(np.int64)
        s_ce += (lse - cl[np.arange(len(pairs)), lbl]).sum()
        d = box_preds[b, cells].astype(np.float64) - gt_boxes[b, ns].astype(np.float64)
        ad = np.abs(d)
        sl1 = np.where(ad < 1.0, 0.5 * d * d, ad - 0.5)
        s_box += sl1.sum()
    return total_pos, s_neg, s_pos, s_ce, s_box


def kernel(cls_logits, obj_logits, box_preds, gt_boxes, gt_labels, gt_masks):
    cls_logits = np.asarray(cls_logits)
    obj_logits = np.asarray(obj_logits)
    box_preds = np.asarray(box_preds)
    B = obj_logits.shape[0]
    assert B == N_CORES, f"expected batch {N_CORES}, got {B}"

    winners = _assign(gt_boxes, gt_labels, gt_masks)

    nc = get_program()
    xbf = np.ascontiguousarray(
        obj_logits.astype(ml_dtypes.bfloat16).reshape(B, P_DIM, COLS)
    )
    in_maps = [{"in_all": xbf[b]} for b in range(B)]
    res = run_bass_kernel_spmd(nc, in_maps, list(range(N_CORES))).results

    # device partials: s_all = sum softplus(obj) over every cell
    #   = sum relu(x) + A*count - A*sum tanh(S*|x| + B)
    nch = len(CHUNKS)
    s_all = 0.0
    for b in range(B):
        o = res[b]["out_all"].astype(np.float64)
        s_all += (
            o[:, :nch].sum() + TANH_A * NUM_CELLS - TANH_A * o[:, nch:].sum()
        )

    total_pos, s_neg, s_pos, s_ce, s_box = _host_positive_sums(
        cls_logits, obj_logits, box_preds, gt_boxes, gt_labels, winners
    )

    M = np.float64(N_CORES * NUM_CELLS)
    positive = np.float64(total_pos)
    negatives = M - positive
    pos_weight = np.maximum(1.0, negatives / (positive + 1e-6))

    obj_loss = (s_all + pos_weight * s_neg - s_pos) / M
    if total_pos > 0:
        cls_loss = s_ce / max(positive, 1.0)
        box_loss = s_box / max(positive * D, 1.0)
    else:
        cls_loss = 0.0
        box_loss = 0.0
    total = obj_loss + CLS_WEIGHT * cls_loss + BOX_WEIGHT * box_loss
    return np.array([total, cls_loss, box_loss, obj_loss], dtype=np.float32)


# revision 3
# speedup vs baseline: 1.0346x; 1.0346x over previous
"""BEV detection loss on 8 Trainium2 NeuronCores — v2.

Strategy (data-parallel over batch, one batch element per core):
  - Only obj_logits needs a full per-cell scan: sum softplus(x) over all
    262144 cells per batch element.  cls/box terms touch <=64 positive
    cells; that tiny gather + math runs on host (as the pack/combine
    already did in v1).
  - Device computes, per core, from x = obj_logits[b] as bf16 [128,2048]:
        Sr = sum relu(x)            (DVE tensor_scalar max+add accum)
        t  = |x|                    (DVE bitwise_and on uint16 view)
        St = sum tanh(s*t + b)      (ACT, one pass, fused accumulate)
    Then sum softplus(x) = Sr + sum ln(1+exp(-t))
                        ~= Sr + A*count - A*St,
    using the 3-parameter fit ln(1+e^-t) ~= A*(1 - tanh(s*t + b)) with
    max pointwise err 1.2e-3 on t>=0 (tanh saturates for large t, so the
    approximation degrades gracefully) -- final loss rel err ~1e-5 on
    normal data, worst case ~7e-4, far inside the 2e-2 gate.
  - bf16 input halves DMA bytes; ACT runs at the same rate either way,
    DVE gets its 2-byte fast mode.
  - DMA: sync + gpsimd queues for the two input chunks (scalar engine
    stays free so the tanh table load starts immediately), sync for the
    tiny [128,4] output of per-partition partial sums.
"""

import sys

import numpy as np

sys.path.insert(0, "/opt/trn_rl_repo")

import concourse.bacc as bacc  # noqa: E402
import concourse.mybir as mybir  # noqa: E402
import concourse.tile as tile  # noqa: E402
from concourse.bass_utils import run_bass_kernel_spmd  # noqa: E402

import ml_dtypes  # noqa: E402

# BEV grid constants (must match the reference)
X_MIN = np.float32(-51.2)
X_MAX = np.float32(51.2)
Y_MIN = np.float32(-51.2)
Y_MAX = np.float32(51.2)
RES = np.float32(0.2)
BEV_W = 512
BEV_H = 512
NUM_CELLS = BEV_W * BEV_H  # 262144
CLS_WEIGHT = np.float64(1.0)
BOX_WEIGHT = np.float64(1.0)

N_CORES = 8
P_DIM = 128
COLS = NUM_CELLS // P_DIM  # 2048
D = 7

# ln(1+e^-t) ~= A*(1 - tanh(S*t + B)) for t >= 0 (max err 1.2e-3)
TANH_A = 1.220978
TANH_S = 0.494342
TANH_B = 0.463907

# chunk column ranges and their input-DMA queue engines; processed in list
# order by the ACT stream.  Small first chunk on the (fast-start) sync queue
# so tanh work begins early; the gpsimd-queue chunk is needed last.
CHUNKS = [(0, 512), (512, 1536), (1536, 2048)]
DMA_ENGINES = ["sync", "gpsimd", "sync"]
WARM_DMA = False  # tiny leading transfer per queue (measured: no benefit)
OUT_W = 2 * len(CHUNKS)  # relu accums + tanh accums per chunk

_CACHE = {}


def _build_program():
    f32 = mybir.dt.float32
    bf16 = mybir.dt.bfloat16
    u16 = mybir.dt.uint16
    AF = mybir.ActivationFunctionType
    ALU = mybir.AluOpType

    nc = bacc.Bacc("TRN2", debug=False, target_bir_lowering=False, num_devices=N_CORES)
    in_all = nc.dram_tensor("in_all", [P_DIM, COLS], bf16, kind="ExternalInput").ap()
    out_all = nc.dram_tensor("out_all", [P_DIM, OUT_W], f32, kind="ExternalOutput").ap()

    with tile.TileContext(nc) as tc:
        with tc.tile_pool(name="p", bufs=1) as p:
            x = p.tile([P_DIM, COLS], bf16)
            if WARM_DMA:
                wdma = p.tile([P_DIM, 2], bf16)
                nc.sync.dma_start(out=wdma[:, 0:1], in_=in_all[:, 0:1])
                nc.gpsimd.dma_start(out=wdma[:, 1:2], in_=in_all[:, 1:2])
            for (lo, hi), eng in zip(CHUNKS, DMA_ENGINES):
                getattr(nc, eng).dma_start(out=x[:, lo:hi], in_=in_all[:, lo:hi])

            out = p.tile([P_DIM, OUT_W], f32)
            nc.vector.memset(out[:], 0.0)
            bias_t = p.tile([P_DIM, 1], f32)
            nc.vector.memset(bias_t[:], float(TANH_B))

            # data-independent warmup: hoists the tanh table load to block
            # start so it overlaps the input DMA (scale=0 -> input unread)
            warm = p.tile([P_DIM, 1], f32)
            nc.scalar.activation(warm[:], warm[:], AF.Tanh, scale=0.0)

            t = p.tile([P_DIM, COLS], bf16)
            y = p.tile([P_DIM, COLS], bf16)
            nch = len(CHUNKS)
            rdead = [p.tile([P_DIM, hi - lo], bf16, name=f"rd{i}", tag=f"rd{i}")
                     for i, (lo, hi) in enumerate(CHUNKS)]

            # |x| per chunk first: unblocks the ACT tanh stream
            for i, (lo, hi) in enumerate(CHUNKS):
                nc.vector.tensor_scalar(
                    t[:, lo:hi].bitcast(u16), x[:, lo:hi].bitcast(u16),
                    0x7FFF, None, op0=ALU.bitwise_and,
                )
            # St: sum tanh(S*t + B) per chunk, fused accumulate
            for i, (lo, hi) in enumerate(CHUNKS):
                nc.scalar.activation(
                    y[:, lo:hi], t[:, lo:hi], AF.Tanh,
                    scale=float(TANH_S), bias=bias_t[:],
                    accum_out=out[:, nch + i : nch + i + 1],
                )
            # Sr: sum relu(x) per chunk (fused elementwise+reduce)
            for i, (lo, hi) in enumerate(CHUNKS):
                nc.vector.tensor_scalar(
                    rdead[i][:], x[:, lo:hi], 0.0, 0.0, op0=ALU.max, op1=ALU.add,
                    accum_out=out[:, i : i + 1],
                )

            nc.sync.dma_start(out=out_all[:], in_=out[:])

    nc.finalize()
    return nc


def get_program():
    if "nc" not in _CACHE:
        _CACHE["nc"] = _build_program()
    return _CACHE["nc"]


def _assign(gt_boxes, gt_labels, gt_masks):
    """Host-side first-come-wins assignment (mirrors reference float32 math).
    Returns per-batch list of (cell, n) winner pairs."""
    B, N = gt_labels.shape
    gb = np.asarray(gt_boxes, dtype=np.float32)
    x = gb[..., 0]
    y = gb[..., 1]
    in_b = (x >= X_MIN) & (x <= X_MAX) & (y >= Y_MIN) & (y <= Y_MAX)
    gx = np.clip(np.floor((x - X_MIN) / RES).astype(np.int32), 0, BEV_W - 1)
    gy = np.clip(np.floor((y - Y_MIN) / RES).astype(np.int32), 0, BEV_H - 1)
    idx = gy * BEV_W + gx  # [B, N]
    valid = (
        (np.asarray(gt_masks, dtype=np.float32) > 0.5)
        & (np.asarray(gt_labels) >= 0)
        & in_b
    )
    winners = []
    for b in range(B):
        seen = set()
        pairs = []
        for n in range(N):
            if not valid[b, n]:
                continue
            cell = int(idx[b, n])
            if cell in seen:
                continue
            seen.add(cell)
            pairs.append((cell, n))
        winners.append(pairs)
    return winners


def _host_positive_sums(cls_logits, obj_logits, box_preds, gt_boxes, gt_labels,
                        winners):
    """Exact f64 math over the <=64 positive cells per batch element."""
    s_neg = 0.0  # sum softplus(-obj) at positives
    s_pos = 0.0  # sum softplus(obj) at positives
    s_ce = 0.0
    s_box = 0.0
    total_pos = 0
    for b, pairs in enumerate(winners):
        if not pairs:
            continue
        cells = np.array([c for c, _ in pairs])
        ns = np.array([n for _, n in pairs])
        total_pos += len(pairs)
        o = obj_logits[b, cells].astype(np.float64)
        s_neg += np.logaddexp(0.0, -o).sum()
        s_pos += np.logaddexp(0.0, o).sum()
        cl = cls_logits[b, cells].astype(np.float64)  # [P, C]
        m = cl.max(axis=1, keepdims=True)
        lse = m[:, 0] + np.log(np.exp(cl - m).sum(axis=1))
        lbl = np.asarray(gt_labels)[b, ns].astype(np.int64)
        s_ce += (lse - cl[np.arange(len(pairs)), lbl]).sum()
        d = box_preds[b, cells].astype(np.float64) - gt_boxes[b, ns].astype(np.float64)
        ad = np.abs(d)
        sl1 = np.where(ad < 1.0, 0.5 * d * d, ad - 0.5)
        s_box += sl1.sum()
    return total_pos, s_neg, s_pos, s_ce, s_box


def kernel(cls_logits, obj_logits, box_preds, gt_boxes, gt_labels, gt_masks):
    cls_logits = np.asarray(cls_logits)
    obj_logits = np.asarray(obj_logits)
    box_preds = np.asarray(box_preds)
    B = obj_logits.shape[0]
    assert B == N_CORES, f"expected batch {N_CORES}, got {B}"

    winners = _assign(gt_boxes, gt_labels, gt_masks)

    nc = get_program()
    xbf = np.ascontiguousarray(
        obj_logits.astype(ml_dtypes.bfloat16).reshape(B, P_DIM, COLS)
    )
    in_maps = [{"in_all": xbf[b]} for b in range(B)]
    res = run_bass_kernel_spmd(nc, in_maps, list(range(N_CORES))).results

    # device partials: s_all = sum softplus(obj) over every cell
    #   = sum relu(x) + A*count - A*sum tanh(S*|x| + B)
    nch = len(CHUNKS)
    s_all = 0.0
    for b in range(B):
        o = res[b]["out_all"].astype(np.float64)
        s_all += (
            o[:, :nch].sum() + TANH_A * NUM_CELLS - TANH_A * o[:, nch:].sum()
        )

    total_pos, s_neg, s_pos, s_ce, s_box = _host_positive_sums(
        cls_logits, obj_logits, box_preds, gt_boxes, gt_labels, winners
    )

    M = np.float64(N_CORES * NUM_CELLS)
    positive = np.float64(total_pos)
    negatives = M - positive
    pos_weight = np.maximum(1.0, negatives / (positive + 1e-6))

    obj_loss = (s_all + pos_weight * s_neg - s_pos) / M
    if total_pos > 0:
        cls_loss = s_ce / max(positive, 1.0)
        box_loss = s_box / max(positive * D, 1.0)
    else:
        cls_loss = 0.0
        box_loss = 0.0
    total = obj_loss + CLS_WEIGHT * cls_loss + BOX_WEIGHT * box_loss
    return np.array([total, cls_loss, box_loss, obj_loss], dtype=np.float32)


# revision 5
# speedup vs baseline: 1.0519x; 1.0167x over previous
"""BEV detection loss on 8 Trainium2 NeuronCores — v2.

Strategy (data-parallel over batch, one batch element per core):
  - Only obj_logits needs a full per-cell scan: sum softplus(x) over all
    262144 cells per batch element.  cls/box terms touch <=64 positive
    cells; that tiny gather + math runs on host (as the pack/combine
    already did in v1).
  - Device computes, per core, from x = obj_logits[b] as bf16 [128,2048]:
        Sr = sum relu(x)            (DVE tensor_scalar max+add accum)
        t  = |x|                    (DVE bitwise_and on uint16 view)
        St = sum tanh(s*t + b)      (ACT, one pass, fused accumulate)
    Then sum softplus(x) = Sr + sum ln(1+exp(-t))
                        ~= Sr + A*count - A*St,
    using the 3-parameter fit ln(1+e^-t) ~= A*(1 - tanh(s*t + b)) with
    max pointwise err 1.2e-3 on t>=0 (tanh saturates for large t, so the
    approximation degrades gracefully) -- final loss rel err ~1e-5 on
    normal data, worst case ~7e-4, far inside the 2e-2 gate.
  - bf16 input halves DMA bytes; ACT runs at the same rate either way,
    DVE gets its 2-byte fast mode.
  - DMA: sync + gpsimd queues for the three input chunks (scalar engine
    stays free so the tanh table load starts immediately), sync for the
    tiny [128,6] output of per-partition partial sums.
"""

import sys

import numpy as np

sys.path.insert(0, "/opt/trn_rl_repo")

import concourse.bacc as bacc  # noqa: E402
import concourse.mybir as mybir  # noqa: E402
import concourse.tile as tile  # noqa: E402
from concourse.bass_utils import run_bass_kernel_spmd  # noqa: E402

import ml_dtypes  # noqa: E402

# BEV grid constants (must match the reference)
X_MIN = np.float32(-51.2)
X_MAX = np.float32(51.2)
Y_MIN = np.float32(-51.2)
Y_MAX = np.float32(51.2)
RES = np.float32(0.2)
BEV_W = 512
BEV_H = 512
NUM_CELLS = BEV_W * BEV_H  # 262144
CLS_WEIGHT = np.float64(1.0)
BOX_WEIGHT = np.float64(1.0)

N_CORES = 8
P_DIM = 128
COLS = NUM_CELLS // P_DIM  # 2048
D = 7

# ln(1+e^-t) ~= A*(1 - tanh(S*t + B)) for t >= 0 (max err 1.2e-3)
TANH_A = 1.220978
TANH_S = 0.494342
TANH_B = 0.463907

# chunk column ranges and their input-DMA queue engines; processed in list
# order by the ACT stream.  Modest first chunk on the sync queue so tanh
# work begins early; the two queues each carry half the bytes so the later
# chunks land while earlier tanh work is still running.
CHUNKS = [(0, 512), (512, 1536), (1536, 2048)]
DMA_ENGINES = ["sync", "gpsimd", "sync"]
WARM_DMA = False  # tiny leading transfer per queue (measured: no benefit)
OUT_W = 2 * len(CHUNKS)  # relu accums + tanh accums per chunk

_CACHE = {}


def _build_program():
    f32 = mybir.dt.float32
    bf16 = mybir.dt.bfloat16
    u16 = mybir.dt.uint16
    AF = mybir.ActivationFunctionType
    ALU = mybir.AluOpType

    nc = bacc.Bacc("TRN2", debug=False, target_bir_lowering=False, num_devices=N_CORES)
    in_all = nc.dram_tensor("in_all", [P_DIM, COLS], bf16, kind="ExternalInput").ap()
    out_all = nc.dram_tensor("out_all", [P_DIM, OUT_W], f32, kind="ExternalOutput").ap()

    with tile.TileContext(nc) as tc:
        with tc.tile_pool(name="p", bufs=1) as p:
            x = p.tile([P_DIM, COLS], bf16)
            if WARM_DMA:
                wdma = p.tile([P_DIM, 2], bf16)
                nc.sync.dma_start(out=wdma[:, 0:1], in_=in_all[:, 0:1])
                nc.gpsimd.dma_start(out=wdma[:, 1:2], in_=in_all[:, 1:2])
            for (lo, hi), eng in zip(CHUNKS, DMA_ENGINES):
                getattr(nc, eng).dma_start(out=x[:, lo:hi], in_=in_all[:, lo:hi])

            out = p.tile([P_DIM, OUT_W], f32)
            nc.vector.memset(out[:], 0.0)
            bias_t = p.tile([P_DIM, 1], f32)
            nc.vector.memset(bias_t[:], float(TANH_B))

            # data-independent warmup: hoists the tanh table load to block
            # start so it overlaps the input DMA (scale=0 -> input unread)
            warm = p.tile([P_DIM, 1], f32)
            nc.scalar.activation(warm[:], warm[:], AF.Tanh, scale=0.0)

            t = p.tile([P_DIM, COLS], bf16)
            y = p.tile([P_DIM, COLS], bf16)
            nch = len(CHUNKS)
            rdead = [p.tile([P_DIM, hi - lo], bf16, name=f"rd{i}", tag=f"rd{i}")
                     for i, (lo, hi) in enumerate(CHUNKS)]

            # |x| per chunk first: unblocks the ACT tanh stream
            for i, (lo, hi) in enumerate(CHUNKS):
                nc.vector.tensor_scalar(
                    t[:, lo:hi].bitcast(u16), x[:, lo:hi].bitcast(u16),
                    0x7FFF, None, op0=ALU.bitwise_and,
                )
            # St: sum tanh(S*t + B) per chunk, fused accumulate
            for i, (lo, hi) in enumerate(CHUNKS):
                nc.scalar.activation(
                    y[:, lo:hi], t[:, lo:hi], AF.Tanh,
                    scale=float(TANH_S), bias=bias_t[:],
                    accum_out=out[:, nch + i : nch + i + 1],
                )
            # Sr: sum relu(x) per chunk (fused elementwise+reduce)
            for i, (lo, hi) in enumerate(CHUNKS):
                nc.vector.tensor_scalar(
                    rdead[i][:], x[:, lo:hi], 0.0, 0.0, op0=ALU.max, op1=ALU.add,
                    accum_out=out[:, i : i + 1],
                )

            nc.sync.dma_start(out=out_all[:], in_=out[:])

    nc.finalize()
    return nc


def get_program():
    if "nc" not in _CACHE:
        _CACHE["nc"] = _build_program()
    return _CACHE["nc"]


def _assign(gt_boxes, gt_labels, gt_masks):
    """Host-side first-come-wins assignment (mirrors reference float32 math).
    Returns per-batch list of (cell, n) winner pairs."""
    B, N = gt_labels.shape
    gb = np.asarray(gt_boxes, dtype=np.float32)
    x = gb[..., 0]
    y = gb[..., 1]
    in_b = (x >= X_MIN) & (x <= X_MAX) & (y >= Y_MIN) & (y <= Y_MAX)
    gx = np.clip(np.floor((x - X_MIN) / RES).astype(np.int32), 0, BEV_W - 1)
    gy = np.clip(np.floor((y - Y_MIN) / RES).astype(np.int32), 0, BEV_H - 1)
    idx = gy * BEV_W + gx  # [B, N]
    valid = (
        (np.asarray(gt_masks, dtype=np.float32) > 0.5)
        & (np.asarray(gt_labels) >= 0)
        & in_b
    )
    winners = []
    for b in range(B):
        seen = set()
        pairs = []
        for n in range(N):
            if not valid[b, n]:
                continue
            cell = int(idx[b, n])
            if cell in seen:
                continue
            seen.add(cell)
            pairs.append((cell, n))
        winners.append(pairs)
    return winners


def _host_positive_sums(cls_logits, obj_logits, box_preds, gt_boxes, gt_labels,
                        winners):
    """Exact f64 math over the <=64 positive cells per batch element."""
    s_neg = 0.0  # sum softplus(-obj) at positives
    s_pos = 0.0  # sum softplus(obj) at positives
    s_ce = 0.0
    s_box = 0.0
    total_pos = 0
    for b, pairs in enumerate(winners):
        if not pairs:
            continue
        cells = np.array([c for c, _ in pairs])
        ns = np.array([n for _, n in pairs])
        total_pos += len(pairs)
        o = obj_logits[b, cells].astype(np.float64)
        s_neg += np.logaddexp(0.0, -o).sum()
        s_pos += np.logaddexp(0.0, o).sum()
        cl = cls_logits[b, cells].astype(np.float64)  # [P, C]
        m = cl.max(axis=1, keepdims=True)
        lse = m[:, 0] + np.log(np.exp(cl - m).sum(axis=1))
        lbl = np.asarray(gt_labels)[b, ns].astype(np.int64)
        s_ce += (lse - cl[np.arange(len(pairs)), lbl]).sum()
        d = box_preds[b, cells].astype(np.float64) - gt_boxes[b, ns].astype(np.float64)
        ad = np.abs(d)
        sl1 = np.where(ad < 1.0, 0.5 * d * d, ad - 0.5)
        s_box += sl1.sum()
    return total_pos, s_neg, s_pos, s_ce, s_box


def kernel(cls_logits, obj_logits, box_preds, gt_boxes, gt_labels, gt_masks):
    cls_logits = np.asarray(cls_logits)
    obj_logits = np.asarray(obj_logits)
    box_preds = np.asarray(box_preds)
    B = obj_logits.shape[0]
    assert B == N_CORES, f"expected batch {N_CORES}, got {B}"

    winners = _assign(gt_boxes, gt_labels, gt_masks)

    nc = get_program()
    xbf = np.ascontiguousarray(
        obj_logits.astype(ml_dtypes.bfloat16).reshape(B, P_DIM, COLS)
    )
    in_maps = [{"in_all": xbf[b]} for b in range(B)]
    res = run_bass_kernel_spmd(nc, in_maps, list(range(N_CORES))).results

    # device partials: s_all = sum softplus(obj) over every cell
    #   = sum relu(x) + A*count - A*sum tanh(S*|x| + B)
    nch = len(CHUNKS)
    s_all = 0.0
    for b in range(B):
        o = res[b]["out_all"].astype(np.float64)
        s_all += (
            o[:, :nch].sum() + TANH_A * NUM_CELLS - TANH_A * o[:, nch:].sum()
        )

    total_pos, s_neg, s_pos, s_ce, s_box = _host_positive_sums(
        cls_logits, obj_logits, box_preds, gt_boxes, gt_labels, winners
    )

    M = np.float64(N_CORES * NUM_CELLS)
    positive = np.float64(total_pos)
    negatives = M - positive
    pos_weight = np.maximum(1.0, negatives / (positive + 1e-6))

    obj_loss = (s_all + pos_weight * s_neg - s_pos) / M
    if total_pos > 0:
        cls_loss = s_ce / max(positive, 1.0)
        box_loss = s_box / max(positive * D, 1.0)
    else:
        cls_loss = 0.0
        box_loss = 0.0
    total = obj_loss + CLS_WEIGHT * cls_loss + BOX_WEIGHT * box_loss
    return np.array([total, cls_loss, box_loss, obj_loss], dtype=np.float32)
